# revision 45
# baseline (speedup 1.0000x reference)
"""Trainium2 Bass kernel for the SE-attention block.

Math (per batch b):
    s[n]   = sum_c x[b,c,n]
    att[c] = sum_n x[b,c,n] * s[n]
    h      = relu(bn(W1 @ att))          (BN folded into scale/bias on host)
    a      = sigmoid(W2 @ h)
    out    = x[b] * a[:, None]

Sharding: data-parallel over batch B=16 across 8 cores (2 batches/core),
weights replicated, no collectives. HBM-traffic bound: ~16.8 MB in +
16.8 MB out per core at ~380 GB/s => ~88 us floor; the schedule's job
is to keep the DMA engines saturated from first load to last store.

Schedule (exact fp32 everywhere):
  - 32 quarter loads ([128,1024] per channel-tile) up front on the SP
    HWDGE ring; consts ride the ACT ring (w1t pre-arranged on host so
    it is one contiguous DMA - a strided load's ~512 descriptors hog
    the shared HWDGE descriptor generator and stall the load ring).
  - per quarter: GpSimd pre-adds tA=x0+x1, tB=x2+x3 (tensor_tensor
    never grabs the DVE/GpSimd shared SBUF port pair), PE broadcasts
    the colsum to all 128 partitions with 2 accumulating ones[128,128]
    matmuls per 512-col chunk, DVE does 4 fused scalar_tensor_tensor
    passes: attq = rowsum(x * sB).
  - gate: att_t = sum_q attq (12 tiny DVE adds), 4 W1 rank-1 matmuls
    into PSUM (kept out of the stream so the PE never waits on DVE),
    relu(bn), 4 W2 matmuls, one batched sigmoid.
  - out = x * a IN-PLACE on the x tiles (no extra buffers, no WAR
    pacing; the tail multiplies all fire the moment the gate is ready).
    b0's multiplies run on ACT only - DVE tensor_scalar can enter
    2-port mode, and whichever of {DVE 2-port op, GpSimd op} starts
    second fully blocks (12-15 us stalls), so DVE multiplies are kept
    clear of b1's GpSimd add stream. b1: 8 DVE / 8 ACT.
  - store rings: ALL b0 stores on the SP ring - its FIFO queues them
    behind the 32 loads so no store byte can steal load bandwidth and
    delay b1's gate (the end is load-finish + gate + 22 us of b1
    stores, so load completion time is everything). b1's stores are
    post-load by construction and split across both rings.
"""

import numpy as np

try:
    import concourse.bass as bass
except ImportError:  # fresh grading dir: repo not on sys.path
    import sys

    for p in ("/opt/trn_rl_repo", "/root/.axon_site/_ro/trn_rl_repo"):
        if p not in sys.path:
            sys.path.insert(0, p)
    import concourse.bass as bass

import concourse.tile as tile
from concourse import bacc, mybir
from concourse.bass_utils import run_bass_kernel_spmd

F32 = mybir.dt.float32
AF = mybir.ActivationFunctionType
ALU = mybir.AluOpType

B, C, N = 16, 512, 4096
CR = 128          # squeeze dim C//4
NCORES = 8
BPC = B // NCORES  # batches per core
P = 128
CT = C // P        # channel tiles per batch
NQ = N // 4        # 1024-wide pipeline quarters
QS = 4             # quarters per batch
NCHUNK = 512       # matmul free-dim max (one psum bank)
BN_EPS = 1e-5

_nc_cache = None


def _build():
    nc = bacc.Bacc(None, target_bir_lowering=False)
    x = nc.declare_dram_parameter("x", [BPC, C, N], F32, isOutput=False)
    w1t = nc.declare_dram_parameter("w1t", [P, CT, CR], F32, isOutput=False)
    w2t = nc.declare_dram_parameter("w2t", [CR, C], F32, isOutput=False)
    bns = nc.declare_dram_parameter("bns", [CR, 1], F32, isOutput=False)
    bnb = nc.declare_dram_parameter("bnb", [CR, 1], F32, isOutput=False)
    y = nc.declare_dram_parameter("y", [BPC, C, N], F32, isOutput=True)

    with tile.TileContext(nc) as tc:
        with (
            tc.tile_pool(name="consts", bufs=1) as consts,
            tc.tile_pool(name="x", bufs=2 * CT * QS) as xpool,
            tc.tile_pool(name="big", bufs=2) as big,
            tc.tile_pool(name="small", bufs=4 * CT) as small,
            tc.tile_pool(name="psum", bufs=2, space="PSUM") as psum,
        ):
            # consts go on the ACT HWDGE ring; the SP ring must start
            # with the first x tile.
            ones128 = consts.tile([P, P], F32)
            nc.vector.memset(ones128, 1.0)
            w1t_sb = consts.tile([P, CT, CR], F32)
            nc.scalar.dma_start(out=w1t_sb, in_=w1t[:])
            w2t_sb = consts.tile([P, C], F32)
            nc.scalar.dma_start(out=w2t_sb, in_=w2t[:])
            bns_sb = consts.tile([P, 1], F32)
            nc.scalar.dma_start(out=bns_sb, in_=bns[:])
            bnb_sb = consts.tile([P, 1], F32)
            nc.scalar.dma_start(out=bnb_sb, in_=bnb[:])

            # Pre-warm ACT tables (relu/sigmoid/copy) on a memset scratch
            # so no table load lands at a gate.
            actscr = consts.tile([P, 1], F32)
            nc.gpsimd.memset(actscr, 0.0)
            scratch_sb = consts.tile([P, 1], F32)
            nc.scalar.activation(scratch_sb, actscr, AF.Relu)
            nc.scalar.activation(scratch_sb, actscr, AF.Sigmoid)
            nc.scalar.mul(scratch_sb, actscr, 1.0)
            nc.scalar.copy(scratch_sb, bns_sb)
            nc.scalar.copy(scratch_sb, bnb_sb)

            # All 32 quarter-loads up front on the SP HWDGE ring in
            # (batch, quarter) order.
            xq = [[[None] * QS for _ in range(CT)] for _ in range(BPC)]
            for b in range(BPC):
                for q in range(QS):
                    for t in range(CT):
                        tile_ = xpool.tile(
                            [P, NQ], F32, tag="x", name=f"x_{b}_{t}_{q}"
                        )
                        nc.sync.dma_start(
                            out=tile_,
                            in_=x[b, t * P : (t + 1) * P, q * NQ : (q + 1) * NQ],
                        )
                        xq[b][t][q] = tile_

            attq_all = [
                [
                    [
                        small.tile([P, 1], F32, tag="attq", name=f"attq_{b}_{q}_{t}")
                        for t in range(CT)
                    ]
                    for q in range(QS)
                ]
                for b in range(BPC)
            ]

            hpsums = [
                psum.tile([P, 1], F32, tag="mlp", name=f"hpsum_{b}")
                for b in range(BPC)
            ]

            def rank1s(b, q):
                # fold W1T @ attq(q) into PSUM; called with a 1-quarter lag
                # (after sb(q+1)'s matmuls) so the PE prefers stream work
                # and these fill its gaps - attq(q) is long ready by then.
                attq = attq_all[b]
                for t in range(CT):
                    nc.tensor.matmul(
                        hpsums[b],
                        w1t_sb[:, t, :],
                        attq[q][t][:],
                        start=(q == 0 and t == 0),
                        stop=(q == QS - 1 and t == CT - 1),
                    )

            def stream_quarter(b, q):
                attq = attq_all[b]
                tA = big.tile([P, NQ], F32, tag="tA", bufs=2, name=f"tA_{b}_{q}")
                tB = big.tile([P, NQ], F32, tag="tB", bufs=2, name=f"tB_{b}_{q}")
                nc.gpsimd.tensor_add(tA, xq[b][0][q], xq[b][1][q])
                nc.gpsimd.tensor_add(tB, xq[b][2][q], xq[b][3][q])
                # sB[m, n] = colsum over all 512 channels broadcast to all
                # 128 partitions: ones[128,128] @ tA + ones @ tB per chunk.
                sb = psum.tile([P, NQ], F32, tag="sb", bufs=3, name=f"sb_{b}_{q}")
                for j in range(NQ // NCHUNK):
                    cols = slice(j * NCHUNK, (j + 1) * NCHUNK)
                    nc.tensor.matmul(
                        sb[:, cols], ones128[:], tA[:, cols],
                        start=True, stop=False,
                    )
                    nc.tensor.matmul(
                        sb[:, cols], ones128[:], tB[:, cols],
                        start=False, stop=True,
                    )
                if q > 0:
                    rank1s(b, q - 1)
                for t in range(CT):
                    junk = big.tile(
                        [P, NQ], F32, tag="junk", bufs=2, name=f"junk_{b}_{q}_{t}"
                    )
                    # fused: junk = (x*1.0)*sb, attq = rowsum(junk)
                    nc.vector.scalar_tensor_tensor(
                        out=junk,
                        in0=xq[b][t][q],
                        scalar=1.0,
                        in1=sb,
                        op0=ALU.mult,
                        op1=ALU.mult,
                        accum_out=attq[q][t],
                    )

            def gate(b):
                hpsum = hpsums[b]
                hb = small.tile([P, 1], F32, tag="hb", name=f"hb_{b}")
                nc.scalar.activation(hb, hpsum, AF.Relu, bias=bnb_sb, scale=bns_sb)
                apsum = psum.tile([P, CT], F32, tag="mlp", name=f"apsum_{b}")
                for t in range(CT):
                    nc.tensor.matmul(
                        apsum[:, t : t + 1],
                        w2t_sb[:, t * P : (t + 1) * P],
                        hb[:],
                        start=True,
                        stop=True,
                    )
                avec = small.tile([P, CT], F32, tag="avec", name=f"avec_{b}")
                nc.scalar.activation(avec, apsum, AF.Sigmoid)
                return avec

            def mult_store(b, i, eng, avec, ring):
                # in-place out = x * a[t], store straight from the x tile
                t, q = i // QS, i % QS
                a_t = avec[:, t : t + 1]
                xv = xq[b][t][q]
                if eng == "dve":
                    nc.vector.tensor_scalar_mul(xv, xv, a_t)
                else:
                    nc.scalar.mul(xv, xv, a_t)
                ring.dma_start(
                    out=y[b, t * P : (t + 1) * P, q * NQ : (q + 1) * NQ],
                    in_=xv,
                )

            for q in range(QS):
                stream_quarter(0, q)
            rank1s(0, QS - 1)
            avec0 = gate(0)
            # avec0c = 0 * x_last + avec0: value-identical to avec0 but
            # data-dependent on the LAST load, so multiplies gated on it
            # finish only after all loads - their ACT-ring stores can
            # never steal load bandwidth (the SP ring is leak-proof by
            # FIFO; this makes the ACT ring leak-proof too).
            avec0c = small.tile([P, CT], F32, tag="avec", name="avec0c")
            for q in range(QS):
                stream_quarter(1, q)
            nc.vector.scalar_tensor_tensor(
                out=avec0c,
                in0=xq[1][CT - 1][QS - 1][:, :CT],
                scalar=0.0,
                in1=avec0,
                op0=ALU.mult,
                op1=ALU.add,
            )
            # b0: all 16 multiplies on ACT (idle during the load phase;
            # alone feeds ~450 GB/s; DVE never runs tensor_scalar while
            # b1's GpSimd adds stream - shared-port exclusive lock).
            # SP-ring half uses avec0 (FIFO holds its stores behind the
            # loads); ACT-ring half is gated on avec0c.
            for i in range(0, 16, 2):
                mult_store(0, i, "act", avec0, nc.sync)
            for i in range(1, 16, 2):
                mult_store(0, i, "act", avec0c, nc.scalar)
            rank1s(1, QS - 1)
            avec1 = gate(1)
            # b1: 8 DVE / 8 ACT, stores split across both rings.
            b1_eng = ["dve", "act", "dve", "act", "dve", "act", "dve", "act",
                      "dve", "act", "dve", "act", "dve", "act", "dve", "act"]
            for i in range(16):
                mult_store(1, i, b1_eng[i], avec1,
                           nc.sync if b1_eng[i] == "dve" else nc.scalar)
    return nc


def _get_nc():
    global _nc_cache
    if _nc_cache is None:
        _nc_cache = _build()
        if not _nc_cache.is_finalized():
            _nc_cache.finalize()
    return _nc_cache


def _host_prep(x, W1, gamma, beta, running_mean, running_var, W2):
    x = np.asarray(x, dtype=np.float32)
    rstd = 1.0 / np.sqrt(np.asarray(running_var, np.float32) + BN_EPS)
    bns = (np.asarray(gamma, np.float32) * rstd).reshape(CR, 1)
    bnb = (
        np.asarray(beta, np.float32)
        - np.asarray(running_mean, np.float32) * bns[:, 0]
    ).reshape(CR, 1)
    # w1t pre-arranged to the SBUF layout [p, t, o]: row (t*P + p) of W1.T
    # lands at partition p, block t -> one contiguous DMA
    w1t = np.ascontiguousarray(
        np.asarray(W1, np.float32).T.reshape(CT, P, CR).transpose(1, 0, 2)
    )  # [P, CT, CR]
    w2t = np.ascontiguousarray(np.asarray(W2, np.float32).T)  # [CR, C]
    in_maps = []
    for c in range(NCORES):
        in_maps.append(
            {
                "x": np.ascontiguousarray(x[c * BPC : (c + 1) * BPC]),
                "w1t": w1t,
                "w2t": w2t,
                "bns": np.ascontiguousarray(bns, np.float32),
                "bnb": np.ascontiguousarray(bnb, np.float32),
            }
        )
    return in_maps


def _run(inputs, **spmd_kwargs):
    in_maps = _host_prep(**inputs)
    res = run_bass_kernel_spmd(
        _get_nc(), in_maps, list(range(NCORES)), **spmd_kwargs
    )
    out = np.concatenate([res.results[c]["y"] for c in range(NCORES)], axis=0)
    return out.astype(np.float32, copy=False), res


def kernel(**inputs):
    out, _ = _run(inputs)
    return out


# revision 49
# speedup vs baseline: 1.1314x; 1.1314x over previous
"""Trainium2 Bass kernel for the SE-attention block.

Math (per batch b):
    s[n]   = sum_c x[b,c,n]
    att[c] = sum_n x[b,c,n] * s[n]
    h      = relu(bn(W1 @ att))          (BN folded into scale/bias on host)
    a      = sigmoid(W2 @ h)
    out    = x[b] * a[:, None]

Sharding: data-parallel over batch B=16 across 8 cores (2 batches/core),
weights replicated, no collectives. HBM-traffic bound: ~16.8 MB in +
16.8 MB out per core at ~380 GB/s => ~88 us floor; the schedule's job
is to keep the DMA engines saturated from first load to last store.

Schedule (exact fp32 everywhere):
  - 32 quarter loads ([128,1024] per channel-tile) up front on the SP
    HWDGE ring; consts ride the ACT ring (w1t pre-arranged on host so
    it is one contiguous DMA - a strided load's ~512 descriptors hog
    the shared HWDGE descriptor generator and stall the load ring).
  - per quarter: GpSimd pre-adds tA=x0+x1, tB=x2+x3 (tensor_tensor
    never grabs the DVE/GpSimd shared SBUF port pair), PE broadcasts
    the colsum to all 128 partitions with 2 accumulating ones[128,128]
    matmuls per 512-col chunk, DVE does 4 fused scalar_tensor_tensor
    passes: attq = rowsum(x * sB).
  - gate: att_t = sum_q attq (12 tiny DVE adds), 4 W1 rank-1 matmuls
    into PSUM (kept out of the stream so the PE never waits on DVE),
    relu(bn), 4 W2 matmuls, one batched sigmoid.
  - out = x * a IN-PLACE on the x tiles (no extra buffers, no WAR
    pacing; the tail multiplies all fire the moment the gate is ready).
    b0's multiplies run on ACT only - DVE tensor_scalar can enter
    2-port mode, and whichever of {DVE 2-port op, GpSimd op} starts
    second fully blocks (12-15 us stalls), so DVE multiplies are kept
    clear of b1's GpSimd add stream. b1: 8 DVE / 8 ACT.
  - store rings: ALL b0 stores on the SP ring - its FIFO queues them
    behind the 32 loads so no store byte can steal load bandwidth and
    delay b1's gate (the end is load-finish + gate + 22 us of b1
    stores, so load completion time is everything). b1's stores are
    post-load by construction and split across both rings.
"""

import numpy as np

try:
    import concourse.bass as bass
except ImportError:  # fresh grading dir: repo not on sys.path
    import sys

    for p in ("/opt/trn_rl_repo", "/root/.axon_site/_ro/trn_rl_repo"):
        if p not in sys.path:
            sys.path.insert(0, p)
    import concourse.bass as bass

import concourse.tile as tile
from concourse import bacc, mybir
from concourse.bass_utils import run_bass_kernel_spmd

F32 = mybir.dt.float32
AF = mybir.ActivationFunctionType
ALU = mybir.AluOpType

B, C, N = 16, 512, 4096
CR = 128          # squeeze dim C//4
NCORES = 8
BPC = B // NCORES  # batches per core
P = 128
CT = C // P        # channel tiles per batch
NQ = N // 4        # 1024-wide pipeline quarters
QS = 4             # quarters per batch
NCHUNK = 512       # matmul free-dim max (one psum bank)
BN_EPS = 1e-5

_nc_cache = None


def _build():
    nc = bacc.Bacc(None, target_bir_lowering=False)
    x = nc.declare_dram_parameter("x", [BPC, C, N], F32, isOutput=False)
    w1t = nc.declare_dram_parameter("w1t", [P, CT, CR], F32, isOutput=False)
    w2t = nc.declare_dram_parameter("w2t", [CR, C], F32, isOutput=False)
    bns = nc.declare_dram_parameter("bns", [CR, 1], F32, isOutput=False)
    bnb = nc.declare_dram_parameter("bnb", [CR, 1], F32, isOutput=False)
    y = nc.declare_dram_parameter("y", [BPC, C, N], F32, isOutput=True)

    with tile.TileContext(nc) as tc:
        with (
            tc.tile_pool(name="consts", bufs=1) as consts,
            tc.tile_pool(name="x", bufs=2 * CT) as xpool,
            tc.tile_pool(name="big", bufs=2) as big,
            tc.tile_pool(name="small", bufs=4 * CT) as small,
            tc.tile_pool(name="psum", bufs=2, space="PSUM") as psum,
        ):
            # consts go on the ACT HWDGE ring; the SP ring must start
            # with the first x tile.
            ones128 = consts.tile([P, P], F32)
            nc.vector.memset(ones128, 1.0)
            w1t_sb = consts.tile([P, CT, CR], F32)
            nc.scalar.dma_start(out=w1t_sb, in_=w1t[:])
            w2t_sb = consts.tile([P, C], F32)
            nc.scalar.dma_start(out=w2t_sb, in_=w2t[:])
            bns_sb = consts.tile([P, 1], F32)
            nc.scalar.dma_start(out=bns_sb, in_=bns[:])
            bnb_sb = consts.tile([P, 1], F32)
            nc.scalar.dma_start(out=bnb_sb, in_=bnb[:])

            # Pre-warm ACT tables (relu/sigmoid/copy) on a memset scratch
            # so no table load lands at a gate.
            actscr = consts.tile([P, 1], F32)
            nc.gpsimd.memset(actscr, 0.0)
            scratch_sb = consts.tile([P, 1], F32)
            nc.scalar.activation(scratch_sb, actscr, AF.Relu)
            nc.scalar.activation(scratch_sb, actscr, AF.Sigmoid)
            nc.scalar.mul(scratch_sb, actscr, 1.0)
            nc.scalar.copy(scratch_sb, bns_sb)
            nc.scalar.copy(scratch_sb, bnb_sb)

            # All 32 quarter-loads up front on the SP HWDGE ring in
            # (batch, quarter) order.
            # x lives in 8 full-row tiles [128, 4096] (one per batch x
            # channel-block) so the multiply is one op and the store is one
            # contiguous 2 MB DMA (16 KB per partition line - best DMA
            # efficiency, and only 8 store DMAs total so the 8 DMAHW
            # completion lanes never throttle the tail). Loads stay at
            # quarter granularity INTO SUBVIEWS so compute still streams.
            xrow = [
                [
                    xpool.tile([P, N], F32, tag="x", bufs=2 * CT, name=f"x_{b}_{t}")
                    for t in range(CT)
                ]
                for b in range(BPC)
            ]
            for b in range(BPC):
                for q in range(QS):
                    for t in range(CT):
                        nc.sync.dma_start(
                            out=xrow[b][t][:, q * NQ : (q + 1) * NQ],
                            in_=x[b, t * P : (t + 1) * P, q * NQ : (q + 1) * NQ],
                        )
            xq = [
                [
                    [xrow[b][t][:, q * NQ : (q + 1) * NQ] for q in range(QS)]
                    for t in range(CT)
                ]
                for b in range(BPC)
            ]

            attq_all = [
                [
                    [
                        small.tile([P, 1], F32, tag="attq", name=f"attq_{b}_{q}_{t}")
                        for t in range(CT)
                    ]
                    for q in range(QS)
                ]
                for b in range(BPC)
            ]

            hpsums = [
                psum.tile([P, 1], F32, tag="mlp", name=f"hpsum_{b}")
                for b in range(BPC)
            ]

            def rank1s(b, q):
                # fold W1T @ attq(q) into PSUM; called with a 1-quarter lag
                # (after sb(q+1)'s matmuls) so the PE prefers stream work
                # and these fill its gaps - attq(q) is long ready by then.
                attq = attq_all[b]
                for t in range(CT):
                    nc.tensor.matmul(
                        hpsums[b],
                        w1t_sb[:, t, :],
                        attq[q][t][:],
                        start=(q == 0 and t == 0),
                        stop=(q == QS - 1 and t == CT - 1),
                    )

            def stream_quarter(b, q):
                attq = attq_all[b]
                tA = big.tile([P, NQ], F32, tag="tA", bufs=2, name=f"tA_{b}_{q}")
                tB = big.tile([P, NQ], F32, tag="tB", bufs=2, name=f"tB_{b}_{q}")
                nc.gpsimd.tensor_add(tA, xq[b][0][q], xq[b][1][q])
                nc.gpsimd.tensor_add(tB, xq[b][2][q], xq[b][3][q])
                # sB[m, n] = colsum over all 512 channels broadcast to all
                # 128 partitions: ones[128,128] @ tA + ones @ tB per chunk.
                sb = psum.tile([P, NQ], F32, tag="sb", bufs=3, name=f"sb_{b}_{q}")
                for j in range(NQ // NCHUNK):
                    cols = slice(j * NCHUNK, (j + 1) * NCHUNK)
                    nc.tensor.matmul(
                        sb[:, cols], ones128[:], tA[:, cols],
                        start=True, stop=False,
                    )
                    nc.tensor.matmul(
                        sb[:, cols], ones128[:], tB[:, cols],
                        start=False, stop=True,
                    )
                if q > 0:
                    rank1s(b, q - 1)
                for t in range(CT):
                    junk = big.tile(
                        [P, NQ], F32, tag="junk", bufs=2, name=f"junk_{b}_{q}_{t}"
                    )
                    # fused: junk = (x*1.0)*sb, attq = rowsum(junk)
                    nc.vector.scalar_tensor_tensor(
                        out=junk,
                        in0=xq[b][t][q],
                        scalar=1.0,
                        in1=sb,
                        op0=ALU.mult,
                        op1=ALU.mult,
                        accum_out=attq[q][t],
                    )

            def gate(b):
                hpsum = hpsums[b]
                hb = small.tile([P, 1], F32, tag="hb", name=f"hb_{b}")
                nc.scalar.activation(hb, hpsum, AF.Relu, bias=bnb_sb, scale=bns_sb)
                apsum = psum.tile([P, CT], F32, tag="mlp", name=f"apsum_{b}")
                for t in range(CT):
                    nc.tensor.matmul(
                        apsum[:, t : t + 1],
                        w2t_sb[:, t * P : (t + 1) * P],
                        hb[:],
                        start=True,
                        stop=True,
                    )
                avec = small.tile([P, CT], F32, tag="avec", name=f"avec_{b}")
                nc.scalar.activation(avec, apsum, AF.Sigmoid)
                return avec

            def mult_store(b, t, eng, avec, ring):
                # in-place out = x * a[t] over the whole [128, 4096] row in
                # one op, then one contiguous 2 MB store from the row tile.
                a_t = avec[:, t : t + 1]
                xv = xrow[b][t]
                if eng == "dve":
                    nc.vector.tensor_scalar_mul(xv, xv, a_t)
                else:
                    nc.scalar.mul(xv, xv, a_t)
                ring.dma_start(
                    out=y[b, t * P : (t + 1) * P, :],
                    in_=xv,
                )

            for q in range(QS):
                stream_quarter(0, q)
            rank1s(0, QS - 1)
            avec0 = gate(0)
            for q in range(QS):
                stream_quarter(1, q)
            # b0: all 4 row-multiplies on ACT (idle during the load phase;
            # DVE never runs tensor_scalar while b1's GpSimd adds stream -
            # shared-port exclusive lock). Stores on the SP ring: FIFO
            # queues them behind the loads, so no store byte can steal
            # load bandwidth and delay b1's gate. The scalar sequencer
            # carries NO store issues until the tail (shared DMAHW
            # completion lanes couple it to the load stream otherwise).
            for t in range(CT):
                mult_store(0, t, "act", avec0, nc.sync)
            rank1s(1, QS - 1)
            avec1 = gate(1)
            # b1: 2 DVE / 2 ACT row-multiplies; stores on the ACT ring,
            # which is idle by then - the two rings drain b0 and b1
            # concurrently in the tail.
            for t in range(CT):
                mult_store(1, t, "dve" if t % 2 == 0 else "act", avec1,
                           nc.scalar)
    return nc


def _get_nc():
    global _nc_cache
    if _nc_cache is None:
        _nc_cache = _build()
        if not _nc_cache.is_finalized():
            _nc_cache.finalize()
    return _nc_cache


def _host_prep(x, W1, gamma, beta, running_mean, running_var, W2):
    x = np.asarray(x, dtype=np.float32)
    rstd = 1.0 / np.sqrt(np.asarray(running_var, np.float32) + BN_EPS)
    bns = (np.asarray(gamma, np.float32) * rstd).reshape(CR, 1)
    bnb = (
        np.asarray(beta, np.float32)
        - np.asarray(running_mean, np.float32) * bns[:, 0]
    ).reshape(CR, 1)
    # w1t pre-arranged to the SBUF layout [p, t, o]: row (t*P + p) of W1.T
    # lands at partition p, block t -> one contiguous DMA
    w1t = np.ascontiguousarray(
        np.asarray(W1, np.float32).T.reshape(CT, P, CR).transpose(1, 0, 2)
    )  # [P, CT, CR]
    w2t = np.ascontiguousarray(np.asarray(W2, np.float32).T)  # [CR, C]
    in_maps = []
    for c in range(NCORES):
        in_maps.append(
            {
                "x": np.ascontiguousarray(x[c * BPC : (c + 1) * BPC]),
                "w1t": w1t,
                "w2t": w2t,
                "bns": np.ascontiguousarray(bns, np.float32),
                "bnb": np.ascontiguousarray(bnb, np.float32),
            }
        )
    return in_maps


def _run(inputs, **spmd_kwargs):
    in_maps = _host_prep(**inputs)
    res = run_bass_kernel_spmd(
        _get_nc(), in_maps, list(range(NCORES)), **spmd_kwargs
    )
    out = np.concatenate([res.results[c]["y"] for c in range(NCORES)], axis=0)
    return out.astype(np.float32, copy=False), res


def kernel(**inputs):
    out, _ = _run(inputs)
    return out
